# revision 1
# baseline (speedup 1.0000x reference)
"""CenterLoss update kernel for Trainium2, 8-core SPMD.

Reference computation (N=16384 samples, C=10000 classes, D=128 dims):
    embeded_labels = labels @ center          # [N,D] gather via one-hot
    diff = embeded_labels - embeded_preds
    grad = (labels.T @ diff) / (counts + 1)   # counts = labels.T @ ones
    out  = center - 0.5 * grad

Because each row of ``labels`` is one-hot, ``labels.T @ labels == diag(counts)``,
so the whole thing collapses to a single pass over ``labels``:

    S      = labels.T @ embeded_preds         # [C,D] per-class sum of preds
    counts = column sums of labels            # [C]
    out    = beta * center + gamma * S
             beta  = 1 - 0.5*counts/(counts+1)
             gamma = 0.5/(counts+1)

The 655MB ``labels`` tensor is streamed through the PE exactly once as the
*moving* matmul operand (computing S.T = preds.T @ labels tile by tile), with
per-partition partial counts accumulated on the vector engine and reduced by
one final PE pass against a ones vector.  Work is data-parallel over N across
8 cores; per-core partial (S.T, counts) blocks are combined with a single
ReduceScatter (add) that also hands each core exactly its C/8 shard, so the
final elementwise update needs no core-dependent addressing.
"""

import numpy as np

N, C, D = 16384, 10000, 128
NCORES = 8
NS = N // NCORES  # 2048 rows per core
CS = C // NCORES  # 1250 classes per core
LR = 0.5
P = 128


def _chunks(width, step=512):
    out = []
    c0 = 0
    while c0 < width:
        out.append((c0, min(step, width - c0)))
        c0 += step
    return out


def build_program(ns=NS, c=C, d=D, ncores=NCORES):
    """Build the SPMD Bass program (identical on every core)."""
    import concourse.bacc as bacc
    import concourse.mybir as mybir
    import concourse.tile as tile
    from concourse.masks import make_identity

    f32 = mybir.dt.float32
    f32r = mybir.dt.float32r
    mult = mybir.AluOpType.mult
    add = mybir.AluOpType.add
    sub = mybir.AluOpType.subtract

    cs = c // ncores
    kt = ns // P            # k-tiles over this core's rows
    gw = 2 * cs             # group width: 2 class-shards per PSUM group
    ng = c // gw            # groups (ncores/2)
    assert ns % P == 0 and c % ncores == 0 and ncores % 2 == 0
    assert gw * 4 <= 5 * 2048, "S.T PSUM tile must fit in 5 banks"
    assert cs * 4 <= 3 * 2048, "counts PSUM tile must fit in 3 banks"

    nc = bacc.Bacc(
        "TRN2",
        target_bir_lowering=False,
        debug=False,
        num_devices=ncores,
    )

    preds = nc.dram_tensor("preds", [ns, d], f32, kind="ExternalInput").ap()
    # labels are one-hot 0/1: declaring them float32r (same bits, trivially
    # rounded) lets plain HWDGE DMAs feed fp32r matmuls at full speed -- the
    # SWDGE cast path bottlenecks on Q7 descriptor generation.
    labels = nc.dram_tensor("labels", [ns, c], f32r, kind="ExternalInput").ap()
    center = nc.dram_tensor("center", [cs, d], f32, kind="ExternalInput").ap()
    out = nc.dram_tensor("out", [cs, d], f32, kind="ExternalOutput").ap()

    nt3 = (cs + P - 1) // P  # phase-3 tiles over the class shard

    with tile.TileContext(nc) as tc:
        with (
            tc.tile_pool(name="const", bufs=1) as const_pool,
            tc.tile_pool(name="dram", bufs=1, space="DRAM") as dram_pool,
        ):
            identity = const_pool.tile([P, P], f32, name="identity")
            make_identity(nc, identity[:])
            ones_col = const_pool.tile([P, 1], f32, name="ones_col")
            nc.vector.memset(ones_col[:], 1.0)

            # preds for this core, as kt stationary [K=128, M=d] tiles.
            # fp32 matmul runs at 4 cycles/row; two fp32r passes over a
            # hi/lo mantissa split run at 2 cycles/row with the same
            # precision (labels are one-hot, so S = sum of selected preds;
            # preds == hi + lo to ~fp32 precision and the PSUM accumulates
            # both passes).
            preds_f32 = const_pool.tile([P, kt * d], f32, name="preds_f32")
            preds_hi = const_pool.tile([P, kt * d], f32r, name="preds_hi")
            preds_lo = const_pool.tile([P, kt * d], f32r, name="preds_lo")
            for t in range(kt):
                nc.sync.dma_start(
                    out=preds_f32[:, t * d:(t + 1) * d],
                    in_=preds[t * P:(t + 1) * P, :],
                )
                # SWDGE cast-DMA rounds fp32 -> fp32r
                nc.gpsimd.dma_start(
                    out=preds_hi[:, t * d:(t + 1) * d],
                    in_=preds[t * P:(t + 1) * P, :],
                )
            nc.vector.tensor_tensor(
                out=preds_lo[:], in0=preds_f32[:], in1=preds_hi[:].bitcast(f32),
                op=sub,
            )

            # center shard, as nt3 [class, d] tiles (class on partitions)
            ctr_sb = const_pool.tile([P, nt3 * d], f32, name="ctr_sb")
            for tt in range(nt3):
                w = min(P, cs - tt * P)
                nc.sync.dma_start(
                    out=ctr_sb[0:w, tt * d:tt * d + d],
                    in_=center[tt * P:tt * P + w, :],
                )

            # per-partition partial counts, accumulated on DVE
            counts_sb = const_pool.tile([P, c], f32, name="counts_sb")

            # partial[j] = [S.T shard j as [d,cs] ; counts shard j] -> block
            # layout matches ReduceScatter's axis-0 split.  The per-rank
            # shard must be a 32-byte multiple (ENCD_DMA_ADDR_ALIGN), so pad
            # the row count; padding rows are never written or read.
            rpad = d + 1
            while (rpad * cs * 4) % 32:
                rpad += 1
            partial = dram_pool.tile([ncores, rpad, cs], f32, name="partial")
            reduced = dram_pool.tile([rpad, cs], f32, name="reduced")
            if rpad > d + 1:
                zpad = const_pool.tile([rpad - d - 1, cs], f32, name="zpad")
                nc.vector.memset(zpad[:], 0.0)
                for j in range(ncores):
                    nc.sync.dma_start(
                        out=partial[j, d + 1:rpad, :], in_=zpad[:]
                    )

            # ---------------- phase 1: stream labels ----------------
            with (
                tc.tile_pool(name="lab", bufs=4) as lab_pool,
                tc.tile_pool(name="psum1", bufs=1, space="PSUM") as psum1,
                tc.tile_pool(name="stage", bufs=2) as stage_pool,
            ):
                for g in range(ng):
                    st_psum = psum1.tile(
                        [d, gw], f32, name=f"st_psum_{g}", tag="st", space="PSUM"
                    )
                    for t in range(kt):
                        lab_t = lab_pool.tile(
                            [P, gw], f32r, name=f"lab_{g}_{t}", tag="lab"
                        )
                        nc.sync.dma_start(
                            out=lab_t[:],
                            in_=labels[t * P:(t + 1) * P,
                                       g * gw:(g + 1) * gw],
                        )
                        for c0, w in _chunks(gw):
                            nc.tensor.matmul(
                                out=st_psum[:, c0:c0 + w],
                                lhsT=preds_hi[:, t * d:(t + 1) * d],
                                rhs=lab_t[:, c0:c0 + w],
                                start=(t == 0),
                                stop=False,
                            )
                            nc.tensor.matmul(
                                out=st_psum[:, c0:c0 + w],
                                lhsT=preds_lo[:, t * d:(t + 1) * d],
                                rhs=lab_t[:, c0:c0 + w],
                                start=False,
                                stop=(t == kt - 1),
                            )
                        if t == 0:
                            nc.vector.tensor_copy(
                                out=counts_sb[:, g * gw:(g + 1) * gw],
                                in_=lab_t[:].bitcast(f32),
                            )
                        else:
                            nc.vector.tensor_add(
                                out=counts_sb[:, g * gw:(g + 1) * gw],
                                in0=counts_sb[:, g * gw:(g + 1) * gw],
                                in1=lab_t[:].bitcast(f32),
                            )
                    st_stage = stage_pool.tile(
                        [d, gw], f32, name=f"st_stage_{g}", tag="stage"
                    )
                    # ACT does the PSUM evacuation so the DVE's serial
                    # counts-add chain never stalls behind it
                    nc.scalar.copy(out=st_stage[:], in_=st_psum[:])
                    for h in range(2):
                        j = 2 * g + h
                        nc.sync.dma_start(
                            out=partial[j, 0:d, :],
                            in_=st_stage[:, h * cs:(h + 1) * cs],
                        )
                        # counts for shard j are final once group g is done:
                        # reduce the 128 partial rows with a ones matmul
                        cnt_psum = psum1.tile(
                            [1, cs], f32, name=f"cnt_psum_{j}", tag="cntp",
                            space="PSUM",
                        )
                        for c0, w in _chunks(cs):
                            nc.tensor.matmul(
                                out=cnt_psum[0:1, c0:c0 + w],
                                lhsT=ones_col[:],
                                rhs=counts_sb[:, j * cs + c0:j * cs + c0 + w],
                                start=True,
                                stop=True,
                            )
                        cnt_stage = stage_pool.tile(
                            [1, cs], f32, name=f"cnt_stage_{j}", tag="cstage"
                        )
                        nc.scalar.copy(out=cnt_stage[:], in_=cnt_psum[:])
                        nc.sync.dma_start(
                            out=partial[j, d:d + 1, :], in_=cnt_stage[0:1, :]
                        )

            # ---------------- phase 2: combine across cores ----------------
            nc.gpsimd.collective_compute(
                "ReduceScatter",
                mybir.AluOpType.add,
                replica_groups=[list(range(ncores))],
                ins=[partial[:].opt()],
                outs=[reduced[:].opt()],
            )

            # ---------------- phase 3: update this core's shard ----------------
            with (
                tc.tile_pool(name="p3", bufs=2) as p3,
                tc.tile_pool(name="psum3", bufs=3, space="PSUM") as psum3,
            ):
                st_sh = const_pool.tile([d, cs], f32, name="st_sh")
                cnt_row = const_pool.tile([1, cs], f32, name="cnt_row")
                nc.sync.dma_start(out=cnt_row[:], in_=reduced[d:d + 1, :])
                # per-tile loads so the first transpose starts immediately
                for tt in range(nt3):
                    w = min(P, cs - tt * P)
                    nc.sync.dma_start(
                        out=st_sh[:, tt * P:tt * P + w],
                        in_=reduced[0:d, tt * P:tt * P + w],
                    )

                for tt in range(nt3):
                    w = min(P, cs - tt * P)
                    trp = psum3.tile([P, d], f32, name=f"trp_{tt}", tag="trp",
                                     space="PSUM")
                    nc.tensor.transpose(
                        out=trp[0:w, 0:d],
                        in_=st_sh[:, tt * P:tt * P + w],
                        identity=identity[:, 0:d],
                    )
                    cntc = psum3.tile([P, 1], f32, name=f"cntc_{tt}", tag="cntc",
                                      space="PSUM")
                    nc.tensor.transpose(
                        out=cntc[0:w, 0:1],
                        in_=cnt_row[0:1, tt * P:tt * P + w],
                        identity=identity[0:1, 0:1],
                    )
                    den = p3.tile([P, 1], f32, name=f"den_{tt}", tag="den")
                    nc.vector.tensor_scalar_add(
                        out=den[0:w, :], in0=cntc[0:w, :], scalar1=1.0
                    )
                    rec = p3.tile([P, 1], f32, name=f"rec_{tt}", tag="rec")
                    nc.vector.reciprocal(out=rec[0:w, :], in_=den[0:w, :])
                    gam = p3.tile([P, 1], f32, name=f"gam_{tt}", tag="gam")
                    nc.vector.tensor_scalar_mul(
                        out=gam[0:w, :], in0=rec[0:w, :], scalar1=0.5
                    )
                    bet = p3.tile([P, 1], f32, name=f"bet_{tt}", tag="bet")
                    nc.vector.tensor_tensor(
                        out=bet[0:w, :], in0=cntc[0:w, :], in1=rec[0:w, :], op=mult
                    )
                    nc.vector.tensor_scalar(
                        out=bet[0:w, :], in0=bet[0:w, :],
                        scalar1=-0.5, scalar2=1.0, op0=mult, op1=add,
                    )
                    o1 = p3.tile([P, d], f32, name=f"o1_{tt}", tag="o1")
                    nc.vector.tensor_scalar_mul(
                        out=o1[0:w, :], in0=ctr_sb[0:w, tt * d:tt * d + d],
                        scalar1=bet[0:w, :],
                    )
                    ou = p3.tile([P, d], f32, name=f"ou_{tt}", tag="ou")
                    nc.vector.scalar_tensor_tensor(
                        out=ou[0:w, :], in0=trp[0:w, 0:d], scalar=gam[0:w, :],
                        in1=o1[0:w, :], op0=mult, op1=add,
                    )
                    nc.sync.dma_start(
                        out=out[tt * P:tt * P + w, :], in_=ou[0:w, 0:d]
                    )

    nc.compile()
    return nc


_PROGRAM = None
LAST_RESULTS = None  # BassKernelResults from the most recent run (for test.py)


def _get_program():
    global _PROGRAM
    if _PROGRAM is None:
        _PROGRAM = build_program()
    return _PROGRAM


def kernel(embeded_preds, labels, center):
    from concourse.bass_utils import run_bass_kernel_spmd

    global LAST_RESULTS
    preds = np.ascontiguousarray(np.asarray(embeded_preds, dtype=np.float32))
    lab = np.ascontiguousarray(np.asarray(labels, dtype=np.float32))
    ctr = np.ascontiguousarray(np.asarray(center, dtype=np.float32))
    assert preds.shape == (N, D) and lab.shape == (N, C) and ctr.shape == (C, D)

    nc = _get_program()
    in_maps = [
        {
            "preds": preds[i * NS:(i + 1) * NS],
            "labels": lab[i * NS:(i + 1) * NS],
            "center": ctr[i * CS:(i + 1) * CS],
        }
        for i in range(NCORES)
    ]
    res = run_bass_kernel_spmd(nc, in_maps, core_ids=list(range(NCORES)))
    LAST_RESULTS = res
    return np.concatenate([res.results[i]["out"] for i in range(NCORES)], axis=0)



# revision 7
# speedup vs baseline: 1.4024x; 1.4024x over previous
"""CenterLoss update kernel for Trainium2, 8-core SPMD.

Reference computation (N=16384 samples, C=10000 classes, D=128 dims):
    embeded_labels = labels @ center          # [N,D] gather via one-hot
    diff = embeded_labels - embeded_preds
    grad = (labels.T @ diff) / (counts + 1)   # counts = labels.T @ ones
    out  = center - 0.5 * grad

Because each row of ``labels`` is one-hot, ``labels.T @ labels == diag(counts)``,
so the whole thing collapses to a single pass over ``labels``:

    S      = labels.T @ embeded_preds         # [C,D] per-class sum of preds
    counts = column sums of labels            # [C]
    out    = beta * center + gamma * S
             beta  = 1 - 0.5*counts/(counts+1)
             gamma = 0.5/(counts+1)

Sharding: by CLASS, not batch.  Core i owns classes [i*1250, (i+1)*1250): it
streams labels[:, shard] (the same 82 MB/core the batch split would read),
additionally reads all of preds (only 8.4 MB), and produces its [1250, 128]
slice of the output with NO inter-core communication at all -- the batch-split
variant needs a serial ReduceScatter of the [C,D] partials which measured
~85us of un-overlappable tail.

The 655MB ``labels`` tensor is streamed through the PE exactly once as the
*moving* matmul operand in a single fp32r pass (fp32r moving rows cost 1
PE cycle at width>=256, same as bf16; the rounding error lands ~1e-3 rel,
far inside the 2e-2 gate).  Label DMAs alternate between the two HWDGE
rings (sync + scalar) so neither ring's fixed costs pace the stream.
Per-partition partial counts accumulate on the DVE and are reduced by one
final PE pass against a ones vector.
"""

import numpy as np

N, C, D = 16384, 10000, 128
NCORES = 8
CS = C // NCORES  # 1250 classes per core
LR = 0.5
P = 128
KT = N // P       # 128 k-tiles over the full batch
PJ = 8            # preds load chunks
PK = KT // PJ     # k-tiles per preds chunk (16)


def _chunks(width, step=512):
    out = []
    c0 = 0
    while c0 < width:
        out.append((c0, min(step, width - c0)))
        c0 += step
    return out


def build_program(n=N, cs=CS, d=D):
    """Build the SPMD Bass program (identical on every core)."""
    import concourse.bacc as bacc
    import concourse.mybir as mybir
    import concourse.tile as tile
    from concourse.masks import make_identity

    f32 = mybir.dt.float32
    f32r = mybir.dt.float32r
    mult = mybir.AluOpType.mult
    add = mybir.AluOpType.add

    kt = n // P
    pk = kt // PJ  # k-tiles per preds chunk
    nt3 = (cs + P - 1) // P  # class tiles for the final update (10)
    assert n % (PJ * P) == 0

    nc = bacc.Bacc(
        "TRN2",
        target_bir_lowering=False,
        debug=False,
        num_devices=NCORES,
    )

    preds = nc.dram_tensor("preds", [n, d], f32, kind="ExternalInput").ap()
    # labels are one-hot 0/1: declaring them float32r (same bits, trivially
    # rounded) lets plain HWDGE DMAs feed fp32r matmuls at full speed.
    labels = nc.dram_tensor("labels", [n, cs], f32r, kind="ExternalInput").ap()
    center = nc.dram_tensor("center", [cs, d], f32, kind="ExternalInput").ap()
    out = nc.dram_tensor("out", [cs, d], f32, kind="ExternalOutput").ap()

    with tile.TileContext(nc) as tc:
        with tc.tile_pool(name="const", bufs=1) as const_pool:
            identity = const_pool.tile([P, P], f32, name="identity")
            make_identity(nc, identity[:])
            ones_col = const_pool.tile([P, 1], f32, name="ones_col")
            nc.vector.memset(ones_col[:], 1.0)

            # all of preds as kt stationary [K=128, M=d] tiles, rounded to
            # fp32r by the DVE (a plain dtype-converting copy) so a single
            # fp32r matmul pass suffices.
            preds_r = const_pool.tile([P, kt * d], f32r, name="preds_r")

            # per-partition partial counts, accumulated on DVE
            cnt_sb = const_pool.tile([P, cs], f32, name="cnt_sb")
            # S.T evacuated from PSUM, and the reduced counts row
            st_sb = const_pool.tile([P, cs], f32, name="st_sb")
            cnt_row = const_pool.tile([1, cs], f32, name="cnt_row")
            # center shard, as nt3 [class, d] tiles (class on partitions)
            ctr_sb = const_pool.tile([P, nt3 * d], f32, name="ctr_sb")

            # ---------------- phase 1: stream labels ----------------
            with (
                tc.tile_pool(name="lab", bufs=12) as lab_pool,
                tc.tile_pool(name="stage", bufs=2) as stage_pool,
                tc.tile_pool(name="psum1", bufs=1, space="PSUM") as psum1,
            ):
                st_psum = psum1.tile([d, cs], f32, name="st_psum", tag="st",
                                     space="PSUM")

                def load_preds_chunk(j):
                    # one preds chunk (pk k-tiles): HWDGE load of the f32
                    # rows, then a dtype-converting DVE copy rounds to f32r.
                    # Emission order IS dataflow order in Tile, so chunk j
                    # must be emitted before the first matmul that reads it.
                    stage = stage_pool.tile([P, pk * d], f32,
                                            name=f"pstage_{j}", tag="stage")
                    nc.scalar.dma_start(
                        out=stage[:].rearrange("p (k d) -> p k d", k=pk),
                        in_=preds[j * pk * P:(j + 1) * pk * P, :]
                            .rearrange("(k p) d -> p k d", p=P),
                    )
                    nc.vector.tensor_copy(
                        out=preds_r[:, j * pk * d:(j + 1) * pk * d],
                        in_=stage[:],
                    )

                load_preds_chunk(0)
                for t in range(kt):
                    lab_t = lab_pool.tile([P, cs], f32r, name=f"lab_{t}",
                                          tag="lab")
                    # alternate the two HWDGE rings so ring fixed costs and
                    # completion latencies overlap across consecutive tiles
                    eng = nc.sync if t % 2 == 0 else nc.scalar
                    eng.dma_start(
                        out=lab_t[:], in_=labels[t * P:(t + 1) * P, :]
                    )
                    for j in range(1, PJ):
                        # interleave the remaining preds chunks into the
                        # scalar ring, each well before its first consumer
                        # (chunk j feeds matmuls t >= j*pk)
                        if t == min(2 * j - 1, j * pk - 1):
                            load_preds_chunk(j)
                    if t == min(17, kt - 1):
                        # center shard, needed only for the tail update
                        for tt in range(nt3):
                            w = min(P, cs - tt * P)
                            nc.scalar.dma_start(
                                out=ctr_sb[0:w, tt * d:tt * d + d],
                                in_=center[tt * P:tt * P + w, :],
                            )
                    for c0, w in _chunks(cs):
                        nc.tensor.matmul(
                            out=st_psum[:, c0:c0 + w],
                            lhsT=preds_r[:, t * d:(t + 1) * d],
                            rhs=lab_t[:, c0:c0 + w],
                            start=(t == 0),
                            stop=(t == kt - 1),
                        )
                    if t == 0:
                        nc.vector.tensor_copy(
                            out=cnt_sb[:], in_=lab_t[:].bitcast(f32)
                        )
                    else:
                        nc.vector.tensor_add(
                            out=cnt_sb[:], in0=cnt_sb[:],
                            in1=lab_t[:].bitcast(f32),
                        )

                # reduce the 128 partial count rows with a ones matmul
                cnt_psum = psum1.tile([1, cs], f32, name="cnt_psum", tag="cnt",
                                      space="PSUM")
                for c0, w in _chunks(cs):
                    nc.tensor.matmul(
                        out=cnt_psum[0:1, c0:c0 + w],
                        lhsT=ones_col[:],
                        rhs=cnt_sb[:, c0:c0 + w],
                        start=True,
                        stop=True,
                    )
                # ACT evacuates both PSUM tiles so phase 3 can reuse the banks
                nc.scalar.copy(out=st_sb[:], in_=st_psum[:])
                nc.scalar.copy(out=cnt_row[:], in_=cnt_psum[:])

            # ---------------- phase 2: update this core's shard ----------------
            with (
                tc.tile_pool(name="p3", bufs=2) as p3,
                tc.tile_pool(name="psum3", bufs=2, space="PSUM") as psum3,
            ):
                for tt in range(nt3):
                    w = min(P, cs - tt * P)
                    trp = psum3.tile([P, d], f32, name=f"trp_{tt}", tag="trp",
                                     space="PSUM")
                    nc.tensor.transpose(
                        out=trp[0:w, 0:d],
                        in_=st_sb[:, tt * P:tt * P + w],
                        identity=identity[:, 0:d],
                    )
                    cntc = psum3.tile([P, 1], f32, name=f"cntc_{tt}", tag="cntc",
                                      space="PSUM")
                    nc.tensor.transpose(
                        out=cntc[0:w, 0:1],
                        in_=cnt_row[0:1, tt * P:tt * P + w],
                        identity=identity[0:1, 0:1],
                    )
                    den = p3.tile([P, 1], f32, name=f"den_{tt}", tag="den")
                    nc.vector.tensor_scalar_add(
                        out=den[0:w, :], in0=cntc[0:w, :], scalar1=1.0
                    )
                    rec = p3.tile([P, 1], f32, name=f"rec_{tt}", tag="rec")
                    nc.vector.reciprocal(out=rec[0:w, :], in_=den[0:w, :])
                    gam = p3.tile([P, 1], f32, name=f"gam_{tt}", tag="gam")
                    nc.vector.tensor_scalar_mul(
                        out=gam[0:w, :], in0=rec[0:w, :], scalar1=0.5
                    )
                    bet = p3.tile([P, 1], f32, name=f"bet_{tt}", tag="bet")
                    nc.vector.tensor_tensor(
                        out=bet[0:w, :], in0=cntc[0:w, :], in1=rec[0:w, :],
                        op=mult,
                    )
                    nc.vector.tensor_scalar(
                        out=bet[0:w, :], in0=bet[0:w, :],
                        scalar1=-0.5, scalar2=1.0, op0=mult, op1=add,
                    )
                    o1 = p3.tile([P, d], f32, name=f"o1_{tt}", tag="o1")
                    nc.vector.tensor_scalar_mul(
                        out=o1[0:w, :], in0=ctr_sb[0:w, tt * d:tt * d + d],
                        scalar1=bet[0:w, :],
                    )
                    ou = p3.tile([P, d], f32, name=f"ou_{tt}", tag="ou")
                    nc.vector.scalar_tensor_tensor(
                        out=ou[0:w, :], in0=trp[0:w, 0:d], scalar=gam[0:w, :],
                        in1=o1[0:w, :], op0=mult, op1=add,
                    )
                    nc.sync.dma_start(
                        out=out[tt * P:tt * P + w, :], in_=ou[0:w, 0:d]
                    )

    nc.compile()
    return nc


_PROGRAM = None
LAST_RESULTS = None  # BassKernelResults from the most recent run (for test.py)


def _get_program():
    global _PROGRAM
    if _PROGRAM is None:
        _PROGRAM = build_program()
    return _PROGRAM


def kernel(embeded_preds, labels, center):
    from concourse.bass_utils import run_bass_kernel_spmd

    global LAST_RESULTS
    preds = np.ascontiguousarray(np.asarray(embeded_preds, dtype=np.float32))
    lab = np.asarray(labels, dtype=np.float32)
    ctr = np.ascontiguousarray(np.asarray(center, dtype=np.float32))
    assert preds.shape == (N, D) and lab.shape == (N, C) and ctr.shape == (C, D)

    nc = _get_program()
    in_maps = [
        {
            "preds": preds,
            "labels": np.ascontiguousarray(lab[:, i * CS:(i + 1) * CS]),
            "center": np.ascontiguousarray(ctr[i * CS:(i + 1) * CS]),
        }
        for i in range(NCORES)
    ]
    res = run_bass_kernel_spmd(nc, in_maps, core_ids=list(range(NCORES)))
    LAST_RESULTS = res
    return np.concatenate([res.results[i]["out"] for i in range(NCORES)], axis=0)
